# revision 41
# baseline (speedup 1.0000x reference)
"""Multi-head attention (B=2, S=2048, D=1024, H=16, Dh=64) on 8 Trainium2
NeuronCores via Bass/Tile.

Sharding: data-parallel over the 2 batches x tensor-parallel over head
groups (16 heads -> 4 groups of 4). Core c = 4*b + g handles batch b and
heads 4g..4g+3 with the matching column/row slices of Wq/Wk/Wv/Wo. Each
core returns its partial output projection (bf16); the host sums the 4
partials per batch and adds bo.

Key packing: masked-out keys contribute nothing to attention (their V_ext
rows are zeroed), and attention is permutation-invariant over keys, so the
host packs only the KEPT keys into `xk` (padded with zeros to a multiple
of 128). With ~10% of keys masked this drops the key-chunk count skc from
16 to 15, shrinking the scores / exp / AV work by 1/16. Queries are NOT
masked, so the full `xt` is still used for the Q and output projections.

Host-side prep (free for the benchmark): x is pre-transposed and pre-cast
to bf16 in device layout [128, 8, S]; weights pre-cast/pre-arranged; Wk
and bk pre-scaled by 1/sqrt(Dh) so no scale is needed in the exp.

Per-core kernel (4 heads = 2 "pairs" of 64-dim heads stacked to fill the
128-partition dim), bf16 matmul datapath with fp32 PSUM accumulation:
  QT   = Wq_g^T xt^T + bq_g             [128 (2 heads x 64), 2 pairs, S]
  KT   = Wk_g^T xk^T + bk_g             [128, 2, SKW] (packed keys)
  V_ext= [(xk Wv_g)*maskf + maskf*bv | maskf]   [kkey, chunk, 4*65] bf16
  per pair, per q-tile (512 queries), per key chunk (128 packed keys):
    scT [128k, 2x512q] = KT_chunk^T @ QT_tile   (2 heads row-packed in PE)
    eT  = exp(scT)                              (one ACT op per chunk)
    ctx_h[65, 512] += V_ext_chunk^T @ eT_h      (row 64 = softmax denom)
  normalize: recip(den) [DVE] -> broadcast [GPSIMD] -> ctxT = ctx*rec [DVE]
  out_partial = ctxT^T @ Wo_g           (PSUM accum over the 2 pairs)

The kernel is one software-pipelined stream over all 8 (pair, q-tile)
attention tiles: exp for chunk kc runs on the Scalar engine while the PE
computes scores(kc+1..) and the AV matmuls lag LAG chunks behind, and all
projection/output matmuls are emitted as scheduled "filler" work inside
the stream so the PE never idles (which would drop its p-state below
2.4 GHz). Input DMAs race on both HWDGE queues, ordered by first use.

PSUM budget (8 banks): scores ring 2x2 banks (double-buffered so exp
reads overlap the next scores), ctx accumulators ring 2x1 (one bank per
head; 65 rows = 64 ctx + denominator), projection scratch ring 2x1
(double-buffered so consecutive filler units never serialize on the DVE
evacuation of the previous one).
"""

import math

import numpy as np
import ml_dtypes

import concourse.bacc as bacc
import concourse.mybir as mybir
import concourse.tile as tile
from concourse.bass_utils import run_bass_kernel_spmd

F32 = mybir.dt.float32
BF16 = mybir.dt.bfloat16
AF = mybir.ActivationFunctionType
ALU = mybir.AluOpType
BF16NP = ml_dtypes.bfloat16

S = 2048
D = 1024
HPC = 4                  # heads per core
DH = 64
PAIRS = 2                # head pairs per core
P = 128
NQT = 4                  # q tiles of 512
QW = 512                 # q tile width
DCH = D // P             # 8 D chunks
SCALE = 1.0 / np.sqrt(DH)

N_CORES = 8


def build(skc):
    """Build the per-core kernel for `skc` 128-wide packed key chunks."""
    assert 13 <= skc <= 16, skc
    SKW = skc * P
    nsl = (SKW + QW - 1) // QW           # key-side projection slices
    kws = [min(QW, SKW - s * QW) for s in range(nsl)]

    nc = bacc.Bacc(None, target_bir_lowering=False, num_swdge_queues=4)

    xt = nc.dram_tensor("xt", [P, DCH, S], BF16, kind="ExternalInput")
    xk = nc.dram_tensor("xk", [P, DCH, SKW], BF16, kind="ExternalInput")
    wq = nc.dram_tensor("wq", [P, DCH, 256], BF16, kind="ExternalInput")
    wk = nc.dram_tensor("wk", [P, DCH, 256], BF16, kind="ExternalInput")
    wv = nc.dram_tensor("wv", [P, DCH, 256], BF16, kind="ExternalInput")
    wo = nc.dram_tensor("wo", [P, PAIRS, D], BF16, kind="ExternalInput")
    bq = nc.dram_tensor("bq", [P, PAIRS], F32, kind="ExternalInput")
    bk = nc.dram_tensor("bk", [P, PAIRS], F32, kind="ExternalInput")
    bv = nc.dram_tensor("bv", [1, 256], F32, kind="ExternalInput")
    maskf = nc.dram_tensor("maskf", [P, skc], F32, kind="ExternalInput")
    out = nc.dram_tensor("out", [S, D], BF16, kind="ExternalOutput")

    with tile.TileContext(nc) as tc:
        with (
            tc.tile_pool(name="persist", bufs=1) as pp,
            tc.tile_pool(name="expp", bufs=6) as ep,
            tc.tile_pool(name="ostage", bufs=5) as op_,
            tc.tile_pool(name="smalls", bufs=4) as sp,
            tc.tile_pool(name="ps_sc", bufs=2, space="PSUM") as ps_sc,
            tc.tile_pool(name="ps_ctx", bufs=2, space="PSUM") as ps_ctx,
            tc.tile_pool(name="ps_w", bufs=2, space="PSUM") as ps_w,
        ):
            # ---- persistent SBUF tensors ----
            maskp = pp.tile([P, skc], F32)
            bq_sb = pp.tile([P, PAIRS], F32)
            bk_sb = pp.tile([P, PAIRS], F32)
            bv_sb = pp.tile([1, 256], F32)
            wq_sb = pp.tile([P, DCH, 256], BF16)
            wk_sb = pp.tile([P, DCH, 256], BF16)
            wv_sb = pp.tile([P, DCH, 256], BF16)
            wo_sb = pp.tile([P, PAIRS, D], BF16)
            bvm_sb = pp.tile([P, skc, 256], BF16)
            xT = pp.tile([P, DCH, S], BF16)
            xK = pp.tile([P, DCH, SKW], BF16)
            QT = pp.tile([P, PAIRS, S], BF16)
            KT = pp.tile([P, PAIRS, SKW], BF16)
            VE = pp.tile([P, skc, HPC * (DH + 1)], BF16)
            ctxT = pp.tile([P, PAIRS, S], BF16)

            # ---- input DMAs on both HWDGE queues, ordered by first use.
            # Each x tensor moves in 512-column slices split into dc-halves
            # so descriptors stay large and both queues share the load.
            def dsl(dst, src, sl, half, eng, w=QW):
                h = (slice(None), slice(4 * half, 4 * half + 4),
                     slice(sl * QW, sl * QW + w))
                eng.dma_start(dst[h], src[h])

            nc.sync.dma_start(wk_sb[:], wk[:, :, :])
            nc.scalar.dma_start(wq_sb[:], wq[:, :, :])
            dsl(xK, xk, 0, 0, nc.sync, kws[0])
            dsl(xK, xk, 0, 1, nc.scalar, kws[0])
            dsl(xT, xt, 0, 0, nc.sync)
            dsl(xT, xt, 0, 1, nc.scalar)
            nc.sync.dma_start(maskp[:], maskf[:, :])
            nc.sync.dma_start(bv_sb[:], bv[:, :])
            nc.scalar.dma_start(wv_sb[:], wv[:, :, :])
            dsl(xK, xk, 1, 0, nc.sync, kws[1])
            nc.sync.dma_start(bq_sb[:], bq[:, :])
            nc.sync.dma_start(bk_sb[:], bk[:, :])
            dsl(xK, xk, 1, 1, nc.scalar, kws[1])
            dsl(xK, xk, 2, 0, nc.sync, kws[2])
            dsl(xK, xk, 2, 1, nc.scalar, kws[2])
            dsl(xK, xk, 3, 0, nc.sync, kws[3])
            dsl(xK, xk, 3, 1, nc.scalar, kws[3])
            dsl(xT, xt, 1, 0, nc.sync)
            dsl(xT, xt, 1, 1, nc.scalar)
            dsl(xT, xt, 2, 0, nc.sync)
            dsl(xT, xt, 2, 1, nc.scalar)
            nc.sync.dma_start(wo_sb[:], wo[:, :, :])
            dsl(xT, xt, 3, 0, nc.sync)
            dsl(xT, xt, 3, 1, nc.scalar)

            ve4 = VE[:].rearrange("p st (h c) -> p st h c", h=HPC)

            # ---- filler units (each emits a small group of PE work) ----
            def v_unit(st):
                def emit():
                    pv = ps_w.tile([P, QW], F32, tag="w", name=f"pv{st}")
                    for dc in range(DCH):
                        nc.tensor.matmul(
                            pv[:, :256],
                            xK[:, dc, st * P : (st + 1) * P],
                            wv_sb[:, dc, :],
                            start=(dc == 0),
                            stop=(dc == DCH - 1),
                        )
                    # ve = (pv * mask) + mask*bv
                    nc.vector.scalar_tensor_tensor(
                        ve4[:, st, :, 0:DH],
                        pv[:, :256].rearrange("p (h c) -> p h c", h=HPC),
                        maskp[:, st : st + 1],
                        bvm_sb[:, st, :].rearrange("p (h c) -> p h c", h=HPC),
                        ALU.mult,
                        ALU.add,
                    )

                return emit, 2048

            def kq_unit(dst, src, w_sb, b_sb, pr, sl, w=QW, off=0):
                """Two 4-matmul halves of a K/Q projection tile."""
                qsl = slice(sl * QW + off, sl * QW + off + w)
                box = {}

                def emit_a():
                    box["pq"] = ps_w.tile([P, QW], F32, tag="w", name=f"pq{pr}_{sl}")
                    for dc in range(4):
                        nc.tensor.matmul(
                            box["pq"][:, :w],
                            w_sb[:, dc, pr * P : (pr + 1) * P],
                            src[:, dc, qsl],
                            start=(dc == 0),
                            stop=False,
                        )

                def emit_b():
                    for dc in range(4, DCH):
                        nc.tensor.matmul(
                            box["pq"][:, :w],
                            w_sb[:, dc, pr * P : (pr + 1) * P],
                            src[:, dc, qsl],
                            start=False,
                            stop=(dc == DCH - 1),
                        )
                    nc.vector.tensor_scalar_add(
                        dst[:, pr, qsl], box["pq"][:, :w], b_sb[:, pr : pr + 1]
                    )

                return (emit_a, w * 4), (emit_b, w * 4)

            ob_tiles = {}

            def out_unit(st, nt, pool=None, scalar_copy=False):
                def emit():
                    if pool is None:
                        po = ps_w.tile([P, QW], F32, tag="w", name=f"po{st}_{nt}")
                    else:
                        po = pool.tile(
                            [P, 2 * QW], F32, tag="sc", name=f"po{st}_{nt}"
                        )[:, :QW]
                    for pr in range(PAIRS):
                        nc.tensor.matmul(
                            po[:],
                            ctxT[:, pr, st * P : (st + 1) * P],
                            wo_sb[:, pr, nt * QW : (nt + 1) * QW],
                            start=(pr == 0),
                            stop=(pr == PAIRS - 1),
                        )
                    if nt == 0:
                        ob_tiles[st] = op_.tile([P, D], BF16, tag="ob", name=f"ob{st}")
                    obt = ob_tiles[st]
                    if scalar_copy:
                        nc.scalar.copy(obt[:, nt * QW : (nt + 1) * QW], po[:])
                    else:
                        nc.vector.tensor_copy(obt[:, nt * QW : (nt + 1) * QW], po[:])
                    nc.sync.dma_start(
                        out[st * P : (st + 1) * P, nt * QW : (nt + 1) * QW],
                        obt[:, nt * QW : (nt + 1) * QW],
                    )

                return emit, 1024

            # ---- attention: one software-pipelined stream, AV lagging
            # LAG chunks behind scores/exp, fillers interleaved by schedule.
            LAG = 3
            deferred = {}

            def normalize(pr, qt, hh, cp, defer=False):
                # reciprocal_approx_fast misbehaves on single-partition
                # tiles: broadcast the PSUM denominator row first, then
                # invert on the broadcast tile.
                qsl = slice(qt * QW, (qt + 1) * QW)
                den = sp.tile([1, QW], F32, tag="den", name=f"den{pr}_{qt}_{hh}")
                nc.vector.tensor_copy(den[:], cp[DH : DH + 1, :])
                denB = sp.tile([DH, QW], F32, tag="denB", name=f"denB{pr}_{qt}_{hh}")
                nc.gpsimd.partition_broadcast(denB[:], den[:])
                recB = sp.tile([DH, QW], F32, tag="recB", name=f"recB{pr}_{qt}_{hh}")
                nc.vector.reciprocal_approx_fast(recB[:], denB[:])
                if defer:
                    deferred[hh] = (cp, recB)
                else:
                    nc.vector.tensor_mul(
                        ctxT[hh * DH : (hh + 1) * DH, pr, qsl], cp[:DH, :], recB[:]
                    )

            def run_stream(schedule):
                jobs = [
                    (pr, qt, kc)
                    for pr in range(PAIRS)
                    for qt in range(NQT)
                    for kc in range(skc)
                ]
                cps_map = {}
                ets = {}
                n_gi = max(len(jobs) + LAG, max(schedule, default=0) + 1)
                for gi in range(n_gi):
                    if gi < len(jobs):
                        pr, qt, kc = jobs[gi]
                        qsl = slice(qt * QW, (qt + 1) * QW)
                        if kc == 0:
                            cps_map[(pr, qt)] = [
                                ps_ctx.tile(
                                    [P, QW], F32, tag="ctx", name=f"ctx{pr}_{qt}_{hh}"
                                )
                                for hh in range(2)
                            ]
                        sc = ps_sc.tile([P, 2 * QW], F32, tag="sc", name=f"sc{gi}")
                        for hh in range(2):
                            nc.tensor.matmul(
                                sc[:, hh * QW : (hh + 1) * QW],
                                KT[hh * DH : (hh + 1) * DH, pr, kc * P : (kc + 1) * P],
                                QT[hh * DH : (hh + 1) * DH, pr, qsl],
                                start=True,
                                stop=True,
                            )
                        et = ep.tile([P, 2 * QW], BF16, tag="et", name=f"et{gi}")
                        nc.scalar.activation(et[:], sc[:], AF.Exp)
                        ets[gi] = et
                    for f in schedule.get(gi, ()):
                        f[0]()
                    if LAG <= gi < len(jobs) + LAG:
                        pr, qt, kk = jobs[gi - LAG]
                        et = ets.pop(gi - LAG)
                        cps = cps_map[(pr, qt)]
                        for hh in range(2):
                            h = 2 * pr + hh
                            nc.tensor.matmul(
                                cps[hh][: DH + 1, :],
                                VE[:, kk, h * (DH + 1) : (h + 1) * (DH + 1)],
                                et[:, hh * QW : (hh + 1) * QW],
                                start=(kk == 0),
                                stop=(kk == skc - 1),
                            )
                            # normalize as soon as this head's accum ends
                            if kk == skc - 1:
                                normalize(pr, qt, hh, cps[hh],
                                          defer=(gi - LAG == len(jobs) - 1))
                        if kk == skc - 1:
                            del cps_map[(pr, qt)]

            # ---- emission schedule ----
            KF = lambda pr, sl: kq_unit(KT, xK, wk_sb, bk_sb, pr, sl, kws[sl])
            QF = lambda pr, qt: kq_unit(QT, xT, wq_sb, bq_sb, pr, qt)

            # prologue: K slice 0 and Q tile 0 of pair 0 only -- the first
            # V tiles go into the stream so a late wv DMA can't delay the
            # first scores.
            for f, _ in KF(0, 0):
                f()
            for f, _ in QF(0, 0):
                f()

            # bvm = maskf (x) bv built on device. Emitted AFTER the prologue
            # projections: the DVE runs in order, so putting these ~5us of
            # mask-plumbing ops (gated by the late maskp DMA) first would
            # make the K00/Q00 bias-adds -- and with them scores(0) -- wait
            # on data only V(0)/AV(0) need.
            bvB = pp.tile([P, 256], F32)
            nc.gpsimd.partition_broadcast(bvB[:], bv_sb[:])
            for st in range(skc):
                nc.vector.tensor_scalar_mul(
                    bvm_sb[:, st, :], bvB[:], maskp[:, st : st + 1]
                )
            # mask columns of V_ext (disjoint from the V column writes)
            nc.vector.tensor_copy(
                ve4[:, :, :, DH : DH + 1],
                maskp[:, :, None, None].to_broadcast([P, skc, HPC, 1]),
            )

            schedule = {}

            def put(gi, *units):
                for u in units:
                    schedule.setdefault(gi, []).append(u)
                    gi += 1

            # tile 0: remaining K slices at 4s-2 (needed by scores at 4s),
            # V tiles greedily at <=1 unit/gi alongside them, <=2 otherwise,
            # all before their AV consumes them (gi j+LAG-1); Q(0,1) last.
            for s2 in range(1, nsl):
                put(4 * s2 - 2, *KF(0, s2))
            cur = 0
            for j in range(skc):
                while len(schedule.get(cur, [])) >= (
                    1 if any(u[1] > 2048 for u in schedule.get(cur, [])) else 2
                ):
                    cur += 1
                assert cur <= j + LAG - 1, (j, cur)
                put(cur, v_unit(j))
            put(skc - 2, *QF(0, 1))
            # tiles 1-3: Q for upcoming tiles, K for pair 1
            put(skc + 8, *QF(0, 2))
            put(2 * skc + 3, *KF(1, 0))
            put(2 * skc + 7, *KF(1, 1))
            put(2 * skc + 11, *QF(0, 3))
            put(3 * skc + 4, *KF(1, 2))
            put(3 * skc + 8, *KF(1, 3))
            put(3 * skc + 12, *QF(1, 0))
            # tiles 4-5: Q for pair 1's later tiles
            put(4 * skc + 4, *QF(1, 1))
            put(4 * skc + 9, *QF(1, 2))
            put(6 * skc + 5, *QF(1, 3))
            # out projections as (1,qt) tiles complete (bases sit a few gi
            # after the tile's normalize chain so the po matmuls never wait
            # on the DVE queue)
            for qt, base in ((0, 5 * skc + 7), (1, 6 * skc + 7), (2, 7 * skc + 5)):
                for i in range(4):
                    st = 4 * qt + i
                    schedule.setdefault(base + 2 * i, []).append(out_unit(st, 0))
                    schedule.setdefault(base + 2 * i + 1, []).append(out_unit(st, 1))

            run_stream(schedule)

            # epilogue: last q-tile's output projection; ctx normalize runs
            # per 128-column slice just ahead of each out projection, and po
            # accumulators rotate through the idle sc ring.
            j = 0
            for st in range(12, 16):
                lo = (st - 12) * P
                for hh in range(2):
                    cp, recB = deferred[hh]
                    nc.vector.tensor_mul(
                        ctxT[hh * DH : (hh + 1) * DH, 1, st * P : (st + 1) * P],
                        cp[:DH, lo : lo + P],
                        recB[:, lo : lo + P],
                    )
                for nt in range(2):
                    out_unit(
                        st, nt, pool=None if j % 3 == 2 else ps_sc,
                        scalar_copy=True,
                    )[0]()
                    j += 1

    nc.finalize()
    return nc


def _pack_keys(xb, maskf_b):
    """Pack kept keys of one batch; returns (xk rows [nk, D], nk)."""
    kept = np.flatnonzero(maskf_b > 0.5)
    return xb[kept], len(kept)


def shard_inputs(x, Wq, bq, Wk, bk, Wv, bv, Wo, bo, mask):
    """Full inputs -> (skc, list of 8 per-core input maps)."""
    maskf = (~np.asarray(mask)).astype(np.float32)  # 1.0 = keep
    x = np.asarray(x, dtype=np.float32)
    Wq, Wk, Wv, Wo = (np.asarray(w, dtype=np.float32) for w in (Wq, Wk, Wv, Wo))
    bq, bk, bv = (np.asarray(b, dtype=np.float32) for b in (bq, bk, bv))

    packed = [_pack_keys(x[b], maskf[b]) for b in range(2)]
    skc = max(13, max(math.ceil(nk / P) for _, nk in packed))
    skc = min(skc, S // P)
    SKW = skc * P

    def dev3(w):  # [1024, 256] -> [128, 8, 256] bf16
        return np.ascontiguousarray(
            w.reshape(DCH, P, 256).transpose(1, 0, 2).astype(BF16NP)
        )

    per_batch = []
    for b in range(2):
        xk_rows, nk = packed[b]
        if nk > SKW:  # mask denser than expected: fall back to unpacked
            xk_rows, nk = x[b], S
        xk_full = np.zeros((SKW, D), np.float32)
        xk_full[:nk] = xk_rows
        xkd = np.ascontiguousarray(
            xk_full.T.reshape(DCH, P, SKW).transpose(1, 0, 2).astype(BF16NP)
        )
        xtd = np.ascontiguousarray(
            x[b].T.reshape(DCH, P, S).transpose(1, 0, 2).astype(BF16NP)
        )
        mp = (np.arange(SKW).reshape(skc, P).T < nk).astype(np.float32)
        per_batch.append((xtd, xkd, np.ascontiguousarray(mp)))

    ins = []
    for c in range(N_CORES):
        b, g = divmod(c, 4)
        cs = slice(g * 256, (g + 1) * 256)
        xtd, xkd, mp = per_batch[b]
        wo_d = np.ascontiguousarray(
            Wo[cs, :].reshape(PAIRS, P, D).transpose(1, 0, 2).astype(BF16NP)
        )
        ins.append(
            {
                "xt": xtd,
                "xk": xkd,
                "wq": dev3(Wq[:, cs]),
                "wk": dev3(Wk[:, cs] * SCALE),
                "wv": dev3(Wv[:, cs]),
                "wo": wo_d,
                "bq": np.ascontiguousarray(bq[cs].reshape(PAIRS, P).T),
                "bk": np.ascontiguousarray(bk[cs].reshape(PAIRS, P).T * SCALE),
                "bv": np.ascontiguousarray(bv[None, cs]),
                "maskf": mp,
            }
        )
    return skc, ins


def gather_outputs(results, bo):
    """8 per-core partial outputs (bf16) -> full (2, S, D) fp32 output."""
    outs = []
    for b in range(2):
        acc = results[4 * b]["out"].astype(np.float32)
        for g in range(1, 4):
            acc += results[4 * b + g]["out"].astype(np.float32)
        outs.append(acc + np.asarray(bo, dtype=np.float32))
    return np.stack(outs, axis=0)


_NC_CACHE = {}


def _get_nc(skc):
    if skc not in _NC_CACHE:
        _NC_CACHE[skc] = build(skc)
    return _NC_CACHE[skc]


def run_sharded(inputs, trace=False, tmpdir=None):
    """Shard, run on cores 0-7, gather. Returns (output, BassKernelResults)."""
    skc, ins = shard_inputs(**inputs)
    nc = _get_nc(skc)
    res = run_bass_kernel_spmd(
        nc, ins, core_ids=list(range(N_CORES)), trace=trace, tmpdir=tmpdir
    )
    full = gather_outputs(res.results, inputs["bo"])
    return full, res


def kernel(**inputs) -> np.ndarray:
    full, _ = run_sharded(inputs, trace=False)
    return full


# revision 43
# speedup vs baseline: 1.1874x; 1.1874x over previous
"""Multi-head attention (B=2, S=2048, D=1024, H=16, Dh=64) on 8 Trainium2
NeuronCores via Bass/Tile.

Sharding: data-parallel over the 2 batches x tensor-parallel over head
groups (16 heads -> 4 groups of 4). Core c = 4*b + g handles batch b and
heads 4g..4g+3 with the matching column/row slices of Wq/Wk/Wv/Wo. Each
core returns its partial output projection (bf16); the host sums the 4
partials per batch and adds bo.

Key packing: masked-out keys contribute nothing to attention (their V_ext
rows are zeroed), and attention is permutation-invariant over keys, so the
host packs only the KEPT keys into `xk` (padded with zeros to a multiple
of 128). With ~10% of keys masked this drops the key-chunk count skc from
16 to 15, shrinking the scores / exp / AV work by 1/16. Queries are NOT
masked, so the full `xt` is still used for the Q and output projections.

Host-side prep (free for the benchmark): x is pre-transposed and pre-cast
to bf16 in device layout [128, 8, S]; weights pre-cast/pre-arranged; Wk
and bk pre-scaled by 1/sqrt(Dh) so no scale is needed in the exp.

Per-core kernel (4 heads = 2 "pairs" of 64-dim heads stacked to fill the
128-partition dim), bf16 matmul datapath with fp32 PSUM accumulation:
  QT   = Wq_g^T xt^T + bq_g             [128 (2 heads x 64), 2 pairs, S]
  KT   = Wk_g^T xk^T + bk_g             [128, 2, SKW] (packed keys)
  V_ext= [(xk Wv_g)*maskf + maskf*bv | maskf]   [kkey, chunk, 4*65] bf16
  per pair, per q-tile (512 queries), per key chunk (128 packed keys):
    scT [128k, 2x512q] = KT_chunk^T @ QT_tile   (2 heads row-packed in PE)
    eT  = exp(scT)                              (one ACT op per chunk)
    ctx_h[65, 512] += V_ext_chunk^T @ eT_h      (row 64 = softmax denom)
  normalize: recip(den) [DVE] -> broadcast [GPSIMD] -> ctxT = ctx*rec [DVE]
  out_partial = ctxT^T @ Wo_g           (PSUM accum over the 2 pairs)

The kernel is one software-pipelined stream over all 8 (pair, q-tile)
attention tiles: exp for chunk kc runs on the Scalar engine while the PE
computes scores(kc+1..) and the AV matmuls lag LAG chunks behind, and all
projection/output matmuls are emitted as scheduled "filler" work inside
the stream so the PE never idles (which would drop its p-state below
2.4 GHz). Input DMAs race on both HWDGE queues, ordered by first use.

PSUM budget (8 banks): scores ring 2x2 banks (double-buffered so exp
reads overlap the next scores), ctx accumulators ring 2x1 (one bank per
head; 65 rows = 64 ctx + denominator), projection scratch ring 2x1
(double-buffered so consecutive filler units never serialize on the DVE
evacuation of the previous one).
"""

import math

import numpy as np
import ml_dtypes

import concourse.bacc as bacc
import concourse.mybir as mybir
import concourse.tile as tile
from concourse.bass_utils import run_bass_kernel_spmd

F32 = mybir.dt.float32
BF16 = mybir.dt.bfloat16
AF = mybir.ActivationFunctionType
ALU = mybir.AluOpType
BF16NP = ml_dtypes.bfloat16

S = 2048
D = 1024
HPC = 4                  # heads per core
DH = 64
PAIRS = 2                # head pairs per core
P = 128
NQT = 4                  # q tiles of 512
QW = 512                 # q tile width
DCH = D // P             # 8 D chunks
SCALE = 1.0 / np.sqrt(DH)

N_CORES = 8


def build(skc):
    """Build the per-core kernel for `skc` 128-wide packed key chunks."""
    assert 13 <= skc <= 16, skc
    SKW = skc * P
    nsl = (SKW + QW - 1) // QW           # key-side projection slices
    kws = [min(QW, SKW - s * QW) for s in range(nsl)]

    nc = bacc.Bacc(None, target_bir_lowering=False, num_swdge_queues=4)

    xt = nc.dram_tensor("xt", [P, DCH, S], BF16, kind="ExternalInput")
    xk = nc.dram_tensor("xk", [P, DCH, SKW], BF16, kind="ExternalInput")
    wq = nc.dram_tensor("wq", [P, DCH, 256], BF16, kind="ExternalInput")
    wk = nc.dram_tensor("wk", [P, DCH, 256], BF16, kind="ExternalInput")
    wv = nc.dram_tensor("wv", [P, DCH, 256], BF16, kind="ExternalInput")
    wo = nc.dram_tensor("wo", [P, PAIRS, D], BF16, kind="ExternalInput")
    bq = nc.dram_tensor("bq", [P, PAIRS], F32, kind="ExternalInput")
    bk = nc.dram_tensor("bk", [P, PAIRS], F32, kind="ExternalInput")
    bv = nc.dram_tensor("bv", [1, 256], F32, kind="ExternalInput")
    maskf = nc.dram_tensor("maskf", [P, skc], F32, kind="ExternalInput")
    out = nc.dram_tensor("out", [S, D], BF16, kind="ExternalOutput")

    with tile.TileContext(nc) as tc:
        with (
            tc.tile_pool(name="persist", bufs=1) as pp,
            tc.tile_pool(name="expp", bufs=6) as ep,
            tc.tile_pool(name="ostage", bufs=5) as op_,
            tc.tile_pool(name="smalls", bufs=4) as sp,
            tc.tile_pool(name="ps_sc", bufs=2, space="PSUM") as ps_sc,
            tc.tile_pool(name="ps_ctx", bufs=2, space="PSUM") as ps_ctx,
            tc.tile_pool(name="ps_w", bufs=2, space="PSUM") as ps_w,
        ):
            # ---- persistent SBUF tensors ----
            maskp = pp.tile([P, skc], F32)
            bq_sb = pp.tile([P, PAIRS], F32)
            bk_sb = pp.tile([P, PAIRS], F32)
            bv_sb = pp.tile([1, 256], F32)
            wq_sb = pp.tile([P, DCH, 256], BF16)
            wk_sb = pp.tile([P, DCH, 256], BF16)
            wv_sb = pp.tile([P, DCH, 256], BF16)
            wo_sb = pp.tile([P, PAIRS, D], BF16)
            bvm_sb = pp.tile([P, skc, 256], BF16)
            xT = pp.tile([P, DCH, S], BF16)
            xK = pp.tile([P, DCH, SKW], BF16)
            QT = pp.tile([P, PAIRS, S], BF16)
            KT = pp.tile([P, PAIRS, SKW], BF16)
            VE = pp.tile([P, skc, HPC * (DH + 1)], BF16)
            ctxT = pp.tile([P, PAIRS, S], BF16)

            # ---- input DMAs on both HWDGE queues, ordered by first use.
            # Each x tensor moves in 512-column slices split into dc-halves
            # so descriptors stay large and both queues share the load.
            def dsl(dst, src, sl, half, eng, w=QW):
                h = (slice(None), slice(4 * half, 4 * half + 4),
                     slice(sl * QW, sl * QW + w))
                eng.dma_start(dst[h], src[h])

            nc.sync.dma_start(wk_sb[:], wk[:, :, :])
            nc.scalar.dma_start(wq_sb[:], wq[:, :, :])
            dsl(xK, xk, 0, 0, nc.sync, kws[0])
            dsl(xK, xk, 0, 1, nc.scalar, kws[0])
            dsl(xT, xt, 0, 0, nc.sync)
            dsl(xT, xt, 0, 1, nc.scalar)
            nc.sync.dma_start(maskp[:], maskf[:, :])
            nc.sync.dma_start(bv_sb[:], bv[:, :])
            nc.scalar.dma_start(wv_sb[:], wv[:, :, :])
            dsl(xK, xk, 1, 0, nc.sync, kws[1])
            nc.sync.dma_start(bq_sb[:], bq[:, :])
            nc.sync.dma_start(bk_sb[:], bk[:, :])
            dsl(xK, xk, 1, 1, nc.scalar, kws[1])
            dsl(xK, xk, 2, 0, nc.sync, kws[2])
            dsl(xK, xk, 2, 1, nc.scalar, kws[2])
            dsl(xK, xk, 3, 0, nc.sync, kws[3])
            dsl(xK, xk, 3, 1, nc.scalar, kws[3])
            dsl(xT, xt, 1, 0, nc.sync)
            dsl(xT, xt, 1, 1, nc.scalar)
            dsl(xT, xt, 2, 0, nc.sync)
            dsl(xT, xt, 2, 1, nc.scalar)
            nc.sync.dma_start(wo_sb[:], wo[:, :, :])
            dsl(xT, xt, 3, 0, nc.sync)
            dsl(xT, xt, 3, 1, nc.scalar)

            ve4 = VE[:].rearrange("p st (h c) -> p st h c", h=HPC)

            # ---- filler units (each emits a small group of PE work) ----
            def v_unit(st):
                def emit():
                    pv = ps_w.tile([P, QW], F32, tag="w", name=f"pv{st}")
                    for dc in range(DCH):
                        nc.tensor.matmul(
                            pv[:, :256],
                            xK[:, dc, st * P : (st + 1) * P],
                            wv_sb[:, dc, :],
                            start=(dc == 0),
                            stop=(dc == DCH - 1),
                        )
                    # ve = (pv * mask) + mask*bv
                    nc.vector.scalar_tensor_tensor(
                        ve4[:, st, :, 0:DH],
                        pv[:, :256].rearrange("p (h c) -> p h c", h=HPC),
                        maskp[:, st : st + 1],
                        bvm_sb[:, st, :].rearrange("p (h c) -> p h c", h=HPC),
                        ALU.mult,
                        ALU.add,
                    )

                return emit, 2048

            def kq_unit(dst, src, w_sb, b_sb, pr, sl, w=QW, off=0):
                """Two 4-matmul halves of a K/Q projection tile."""
                qsl = slice(sl * QW + off, sl * QW + off + w)
                box = {}

                def emit_a():
                    box["pq"] = ps_w.tile([P, QW], F32, tag="w", name=f"pq{pr}_{sl}")
                    for dc in range(4):
                        nc.tensor.matmul(
                            box["pq"][:, :w],
                            w_sb[:, dc, pr * P : (pr + 1) * P],
                            src[:, dc, qsl],
                            start=(dc == 0),
                            stop=False,
                        )

                def emit_b():
                    for dc in range(4, DCH):
                        nc.tensor.matmul(
                            box["pq"][:, :w],
                            w_sb[:, dc, pr * P : (pr + 1) * P],
                            src[:, dc, qsl],
                            start=False,
                            stop=(dc == DCH - 1),
                        )
                    nc.vector.tensor_scalar_add(
                        dst[:, pr, qsl], box["pq"][:, :w], b_sb[:, pr : pr + 1]
                    )

                return (emit_a, w * 4), (emit_b, w * 4)

            ob_tiles = {}

            def out_unit(st, nt, pool=None, scalar_copy=False):
                def emit():
                    if pool is None:
                        po = ps_w.tile([P, QW], F32, tag="w", name=f"po{st}_{nt}")
                    else:
                        po = pool.tile(
                            [P, 2 * QW], F32, tag="sc", name=f"po{st}_{nt}"
                        )[:, :QW]
                    for pr in range(PAIRS):
                        nc.tensor.matmul(
                            po[:],
                            ctxT[:, pr, st * P : (st + 1) * P],
                            wo_sb[:, pr, nt * QW : (nt + 1) * QW],
                            start=(pr == 0),
                            stop=(pr == PAIRS - 1),
                        )
                    if nt == 0:
                        ob_tiles[st] = op_.tile([P, D], BF16, tag="ob", name=f"ob{st}")
                    obt = ob_tiles[st]
                    if scalar_copy:
                        nc.scalar.copy(obt[:, nt * QW : (nt + 1) * QW], po[:])
                    else:
                        nc.vector.tensor_copy(obt[:, nt * QW : (nt + 1) * QW], po[:])
                    nc.sync.dma_start(
                        out[st * P : (st + 1) * P, nt * QW : (nt + 1) * QW],
                        obt[:, nt * QW : (nt + 1) * QW],
                    )

                return emit, 1024

            # ---- attention: one software-pipelined stream, AV lagging
            # LAG chunks behind scores/exp, fillers interleaved by schedule.
            LAG = 3
            deferred = {}

            def normalize(pr, qt, hh, cp, defer=False):
                # reciprocal_approx_fast misbehaves on single-partition
                # tiles: broadcast the PSUM denominator row first, then
                # invert on the broadcast tile.
                qsl = slice(qt * QW, (qt + 1) * QW)
                den = sp.tile([1, QW], F32, tag="den", name=f"den{pr}_{qt}_{hh}")
                nc.vector.tensor_copy(den[:], cp[DH : DH + 1, :])
                denB = sp.tile([DH, QW], F32, tag="denB", name=f"denB{pr}_{qt}_{hh}")
                nc.gpsimd.partition_broadcast(denB[:], den[:])
                recB = sp.tile([DH, QW], F32, tag="recB", name=f"recB{pr}_{qt}_{hh}")
                nc.vector.reciprocal_approx_fast(recB[:], denB[:])
                if defer:
                    deferred[hh] = (cp, recB)
                else:
                    nc.vector.tensor_mul(
                        ctxT[hh * DH : (hh + 1) * DH, pr, qsl], cp[:DH, :], recB[:]
                    )

            def run_stream(schedule):
                jobs = [
                    (pr, qt, kc)
                    for pr in range(PAIRS)
                    for qt in range(NQT)
                    for kc in range(skc)
                ]
                cps_map = {}
                ets = {}
                n_gi = max(len(jobs) + LAG, max(schedule, default=0) + 1)
                for gi in range(n_gi):
                    if gi < len(jobs):
                        pr, qt, kc = jobs[gi]
                        qsl = slice(qt * QW, (qt + 1) * QW)
                        if kc == 0:
                            cps_map[(pr, qt)] = [
                                ps_ctx.tile(
                                    [P, QW], F32, tag="ctx", name=f"ctx{pr}_{qt}_{hh}"
                                )
                                for hh in range(2)
                            ]
                        sc = ps_sc.tile([P, 2 * QW], F32, tag="sc", name=f"sc{gi}")
                        for hh in range(2):
                            nc.tensor.matmul(
                                sc[:, hh * QW : (hh + 1) * QW],
                                KT[hh * DH : (hh + 1) * DH, pr, kc * P : (kc + 1) * P],
                                QT[hh * DH : (hh + 1) * DH, pr, qsl],
                                start=True,
                                stop=True,
                            )
                        et = ep.tile([P, 2 * QW], BF16, tag="et", name=f"et{gi}")
                        nc.scalar.activation(et[:], sc[:], AF.Exp)
                        ets[gi] = et
                    for f in schedule.get(gi, ()):
                        f[0]()
                    if LAG <= gi < len(jobs) + LAG:
                        pr, qt, kk = jobs[gi - LAG]
                        et = ets.pop(gi - LAG)
                        cps = cps_map[(pr, qt)]
                        for hh in range(2):
                            h = 2 * pr + hh
                            nc.tensor.matmul(
                                cps[hh][: DH + 1, :],
                                VE[:, kk, h * (DH + 1) : (h + 1) * (DH + 1)],
                                et[:, hh * QW : (hh + 1) * QW],
                                start=(kk == 0),
                                stop=(kk == skc - 1),
                            )
                            # normalize as soon as this head's accum ends
                            if kk == skc - 1:
                                normalize(pr, qt, hh, cps[hh],
                                          defer=(gi - LAG == len(jobs) - 1))
                        if kk == skc - 1:
                            del cps_map[(pr, qt)]

            # ---- emission schedule ----
            KF = lambda pr, sl: kq_unit(KT, xK, wk_sb, bk_sb, pr, sl, kws[sl])
            QF = lambda pr, qt: kq_unit(QT, xT, wq_sb, bq_sb, pr, qt)

            # prologue: K slice 0 and Q tile 0 of pair 0 only -- the first
            # V tiles go into the stream so a late wv DMA can't delay the
            # first scores.
            for f, _ in KF(0, 0):
                f()
            for f, _ in QF(0, 0):
                f()

            # bvm = maskf (x) bv built on device. Emitted AFTER the prologue
            # projections: the DVE runs in order, so putting these ~5us of
            # mask-plumbing ops (gated by the late maskp DMA) first would
            # make the K00/Q00 bias-adds -- and with them scores(0) -- wait
            # on data only V(0)/AV(0) need.
            bvB = pp.tile([P, 256], F32)
            nc.gpsimd.partition_broadcast(bvB[:], bv_sb[:])
            for st in range(skc):
                nc.vector.tensor_scalar_mul(
                    bvm_sb[:, st, :], bvB[:], maskp[:, st : st + 1]
                )
            # mask columns of V_ext (disjoint from the V column writes)
            nc.vector.tensor_copy(
                ve4[:, :, :, DH : DH + 1],
                maskp[:, :, None, None].to_broadcast([P, skc, HPC, 1]),
            )

            schedule = {}

            def put(gi, *units):
                for u in units:
                    schedule.setdefault(gi, []).append(u)
                    gi += 1

            # tile 0: remaining K slices at 4s-2 (needed by scores at 4s),
            # V tiles greedily at <=1 unit/gi alongside them, <=2 otherwise,
            # all before their AV consumes them (gi j+LAG-1); Q(0,1) last.
            for s2 in range(1, nsl):
                put(4 * s2 - 2, *KF(0, s2))
            cur = 0
            for j in range(skc):
                while len(schedule.get(cur, [])) >= (
                    1 if any(u[1] > 2048 for u in schedule.get(cur, [])) else 2
                ):
                    cur += 1
                assert cur <= j + LAG - 1, (j, cur)
                put(cur, v_unit(j))
            put(skc - 2, *QF(0, 1))
            # tiles 1-3: Q for upcoming tiles, K for pair 1
            put(skc + 8, *QF(0, 2))
            put(2 * skc + 3, *KF(1, 0))
            put(2 * skc + 7, *KF(1, 1))
            put(2 * skc + 11, *QF(0, 3))
            put(3 * skc + 4, *KF(1, 2))
            put(3 * skc + 8, *KF(1, 3))
            put(3 * skc + 12, *QF(1, 0))
            # tiles 4-5: Q for pair 1's later tiles
            put(4 * skc + 4, *QF(1, 1))
            put(4 * skc + 9, *QF(1, 2))
            put(6 * skc + 5, *QF(1, 3))
            # out projections as (1,qt) tiles complete (bases sit a few gi
            # after the tile's normalize chain so the po matmuls never wait
            # on the DVE queue)
            for qt, base in ((0, 5 * skc + 7), (1, 6 * skc + 7), (2, 7 * skc + 5)):
                for i in range(4):
                    st = 4 * qt + i
                    schedule.setdefault(base + 2 * i, []).append(out_unit(st, 0))
                    schedule.setdefault(base + 2 * i + 1, []).append(out_unit(st, 1))

            run_stream(schedule)

            # epilogue: last q-tile's output projection; ctx normalize runs
            # per 128-column slice just ahead of each out projection, and po
            # accumulators rotate through the idle sc ring.
            j = 0
            for st in range(12, 16):
                lo = (st - 12) * P
                for hh in range(2):
                    cp, recB = deferred[hh]
                    nc.vector.tensor_mul(
                        ctxT[hh * DH : (hh + 1) * DH, 1, st * P : (st + 1) * P],
                        cp[:DH, lo : lo + P],
                        recB[:, lo : lo + P],
                    )
                for nt in range(2):
                    out_unit(
                        st, nt, pool=None if j % 3 == 2 else ps_sc,
                        scalar_copy=True,
                    )[0]()
                    j += 1

    nc.finalize()
    return nc


def _pack_keys(xb, maskf_b):
    """Pack kept keys of one batch; returns (xk rows [nk, D], nk)."""
    kept = np.flatnonzero(maskf_b > 0.5)
    return xb[kept], len(kept)


def shard_inputs(x, Wq, bq, Wk, bk, Wv, bv, Wo, bo, mask):
    """Full inputs -> (skc, list of 8 per-core input maps)."""
    maskf = (~np.asarray(mask)).astype(np.float32)  # 1.0 = keep
    x = np.asarray(x, dtype=np.float32)
    Wq, Wk, Wv, Wo = (np.asarray(w, dtype=np.float32) for w in (Wq, Wk, Wv, Wo))
    bq, bk, bv = (np.asarray(b, dtype=np.float32) for b in (bq, bk, bv))

    packed = [_pack_keys(x[b], maskf[b]) for b in range(2)]
    skc = max(13, max(math.ceil(nk / P) for _, nk in packed))
    skc = min(skc, S // P)
    SKW = skc * P

    def dev3(w):  # [1024, 256] -> [128, 8, 256] bf16
        return np.ascontiguousarray(
            w.reshape(DCH, P, 256).transpose(1, 0, 2).astype(BF16NP)
        )

    per_batch = []
    for b in range(2):
        xk_rows, nk = packed[b]
        if nk > SKW:  # mask denser than expected: fall back to unpacked
            xk_rows, nk = x[b], S
        xk_full = np.zeros((SKW, D), np.float32)
        xk_full[:nk] = xk_rows
        xkd = np.ascontiguousarray(
            xk_full.T.reshape(DCH, P, SKW).transpose(1, 0, 2).astype(BF16NP)
        )
        xtd = np.ascontiguousarray(
            x[b].T.reshape(DCH, P, S).transpose(1, 0, 2).astype(BF16NP)
        )
        mp = (np.arange(SKW).reshape(skc, P).T < nk).astype(np.float32)
        per_batch.append((xtd, xkd, np.ascontiguousarray(mp)))

    ins = []
    for c in range(N_CORES):
        b, g = divmod(c, 4)
        cs = slice(g * 256, (g + 1) * 256)
        xtd, xkd, mp = per_batch[b]
        wo_d = np.ascontiguousarray(
            Wo[cs, :].reshape(PAIRS, P, D).transpose(1, 0, 2).astype(BF16NP)
        )
        ins.append(
            {
                "xt": xtd,
                "xk": xkd,
                "wq": dev3(Wq[:, cs]),
                "wk": dev3(Wk[:, cs] * SCALE),
                "wv": dev3(Wv[:, cs]),
                "wo": wo_d,
                "bq": np.ascontiguousarray(bq[cs].reshape(PAIRS, P).T),
                "bk": np.ascontiguousarray(bk[cs].reshape(PAIRS, P).T * SCALE),
                "bv": np.ascontiguousarray(bv[None, cs]),
                "maskf": mp,
            }
        )
    return skc, ins


def gather_outputs(results, bo):
    """8 per-core partial outputs (bf16) -> full (2, S, D) fp32 output."""
    outs = []
    for b in range(2):
        acc = results[4 * b]["out"].astype(np.float32)
        for g in range(1, 4):
            acc += results[4 * b + g]["out"].astype(np.float32)
        outs.append(acc + np.asarray(bo, dtype=np.float32))
    return np.stack(outs, axis=0)


_NC_CACHE = {}


def _get_nc(skc):
    if skc not in _NC_CACHE:
        _NC_CACHE[skc] = build(skc)
    return _NC_CACHE[skc]


def run_sharded(inputs, trace=False, tmpdir=None):
    """Shard, run on cores 0-7, gather. Returns (output, BassKernelResults)."""
    skc, ins = shard_inputs(**inputs)
    nc = _get_nc(skc)
    res = run_bass_kernel_spmd(
        nc, ins, core_ids=list(range(N_CORES)), trace=trace, tmpdir=tmpdir
    )
    full = gather_outputs(res.results, inputs["bo"])
    return full, res


def kernel(**inputs) -> np.ndarray:
    full, _ = run_sharded(inputs, trace=False)
    return full


# revision 44
# speedup vs baseline: 1.1917x; 1.0036x over previous
"""Multi-head attention (B=2, S=2048, D=1024, H=16, Dh=64) on 8 Trainium2
NeuronCores via Bass/Tile.

Sharding: data-parallel over the 2 batches x tensor-parallel over head
groups (16 heads -> 4 groups of 4). Core c = 4*b + g handles batch b and
heads 4g..4g+3 with the matching column/row slices of Wq/Wk/Wv/Wo. Each
core returns its partial output projection (bf16); the host sums the 4
partials per batch and adds bo.

Key packing: masked-out keys contribute nothing to attention (their V_ext
rows are zeroed), and attention is permutation-invariant over keys, so the
host packs only the KEPT keys into `xk` (padded with zeros to a multiple
of 128). With ~10% of keys masked this drops the key-chunk count skc from
16 to 15, shrinking the scores / exp / AV work by 1/16. Queries are NOT
masked, so the full `xt` is still used for the Q and output projections.

Host-side prep (free for the benchmark): x is pre-transposed and pre-cast
to bf16 in device layout [128, 8, S]; weights pre-cast/pre-arranged; Wk
and bk pre-scaled by 1/sqrt(Dh) so no scale is needed in the exp.

Per-core kernel (4 heads = 2 "pairs" of 64-dim heads stacked to fill the
128-partition dim), bf16 matmul datapath with fp32 PSUM accumulation:
  QT   = Wq_g^T xt^T + bq_g             [128 (2 heads x 64), 2 pairs, S]
  KT   = Wk_g^T xk^T + bk_g             [128, 2, SKW] (packed keys)
  V_ext= [(xk Wv_g)*maskf + maskf*bv | maskf]   [kkey, chunk, 4*65] bf16
  per pair, per q-tile (512 queries), per key chunk (128 packed keys):
    scT [128k, 2x512q] = KT_chunk^T @ QT_tile   (2 heads row-packed in PE)
    eT  = exp(scT)                              (one ACT op per chunk)
    ctx_h[65, 512] += V_ext_chunk^T @ eT_h      (row 64 = softmax denom)
  normalize: recip(den) [DVE] -> broadcast [GPSIMD] -> ctxT = ctx*rec [DVE]
  out_partial = ctxT^T @ Wo_g           (PSUM accum over the 2 pairs)

The kernel is one software-pipelined stream over all 8 (pair, q-tile)
attention tiles: exp for chunk kc runs on the Scalar engine while the PE
computes scores(kc+1..) and the AV matmuls lag LAG chunks behind, and all
projection/output matmuls are emitted as scheduled "filler" work inside
the stream so the PE never idles (which would drop its p-state below
2.4 GHz). Input DMAs race on both HWDGE queues, ordered by first use.

PSUM budget (8 banks): scores ring 2x2 banks (double-buffered so exp
reads overlap the next scores), ctx accumulators ring 2x1 (one bank per
head; 65 rows = 64 ctx + denominator), projection scratch ring 2x1
(double-buffered so consecutive filler units never serialize on the DVE
evacuation of the previous one).
"""

import math

import numpy as np
import ml_dtypes

import concourse.bacc as bacc
import concourse.mybir as mybir
import concourse.tile as tile
from concourse.bass_utils import run_bass_kernel_spmd

F32 = mybir.dt.float32
BF16 = mybir.dt.bfloat16
AF = mybir.ActivationFunctionType
ALU = mybir.AluOpType
BF16NP = ml_dtypes.bfloat16

S = 2048
D = 1024
HPC = 4                  # heads per core
DH = 64
PAIRS = 2                # head pairs per core
P = 128
NQT = 4                  # q tiles of 512
QW = 512                 # q tile width
DCH = D // P             # 8 D chunks
SCALE = 1.0 / np.sqrt(DH)

N_CORES = 8


def build(skc):
    """Build the per-core kernel for `skc` 128-wide packed key chunks."""
    assert 13 <= skc <= 16, skc
    SKW = skc * P
    nsl = (SKW + QW - 1) // QW           # key-side projection slices
    kws = [min(QW, SKW - s * QW) for s in range(nsl)]

    nc = bacc.Bacc(None, target_bir_lowering=False, num_swdge_queues=4)

    xt = nc.dram_tensor("xt", [P, DCH, S], BF16, kind="ExternalInput")
    xk = nc.dram_tensor("xk", [P, DCH, SKW], BF16, kind="ExternalInput")
    wq = nc.dram_tensor("wq", [P, DCH, 256], BF16, kind="ExternalInput")
    wk = nc.dram_tensor("wk", [P, DCH, 256], BF16, kind="ExternalInput")
    wv = nc.dram_tensor("wv", [P, DCH, 256], BF16, kind="ExternalInput")
    wo = nc.dram_tensor("wo", [P, PAIRS, D], BF16, kind="ExternalInput")
    bq = nc.dram_tensor("bq", [P, PAIRS], F32, kind="ExternalInput")
    bk = nc.dram_tensor("bk", [P, PAIRS], F32, kind="ExternalInput")
    bv = nc.dram_tensor("bv", [1, 256], F32, kind="ExternalInput")
    maskf = nc.dram_tensor("maskf", [P, skc], F32, kind="ExternalInput")
    out = nc.dram_tensor("out", [S, D], BF16, kind="ExternalOutput")

    with tile.TileContext(nc) as tc:
        with (
            tc.tile_pool(name="persist", bufs=1) as pp,
            tc.tile_pool(name="expp", bufs=6) as ep,
            tc.tile_pool(name="ostage", bufs=5) as op_,
            tc.tile_pool(name="smalls", bufs=4) as sp,
            tc.tile_pool(name="ps_sc", bufs=2, space="PSUM") as ps_sc,
            tc.tile_pool(name="ps_ctx", bufs=2, space="PSUM") as ps_ctx,
            tc.tile_pool(name="ps_w", bufs=2, space="PSUM") as ps_w,
        ):
            # ---- persistent SBUF tensors ----
            maskp = pp.tile([P, skc], F32)
            bq_sb = pp.tile([P, PAIRS], F32)
            bk_sb = pp.tile([P, PAIRS], F32)
            bv_sb = pp.tile([1, 256], F32)
            wq_sb = pp.tile([P, DCH, 256], BF16)
            wk_sb = pp.tile([P, DCH, 256], BF16)
            wv_sb = pp.tile([P, DCH, 256], BF16)
            wo_sb = pp.tile([P, PAIRS, D], BF16)
            bvm_sb = pp.tile([P, skc, 256], BF16)
            xT = pp.tile([P, DCH, S], BF16)
            xK = pp.tile([P, DCH, SKW], BF16)
            QT = pp.tile([P, PAIRS, S], BF16)
            KT = pp.tile([P, PAIRS, SKW], BF16)
            VE = pp.tile([P, skc, HPC * (DH + 1)], BF16)
            ctxT = pp.tile([P, PAIRS, S], BF16)

            # ---- input DMAs on both HWDGE queues, ordered by first use.
            # Each x tensor moves in 512-column slices split into dc-halves
            # so descriptors stay large and both queues share the load.
            def dsl(dst, src, sl, half, eng, w=QW):
                h = (slice(None), slice(4 * half, 4 * half + 4),
                     slice(sl * QW, sl * QW + w))
                eng.dma_start(dst[h], src[h])

            nc.sync.dma_start(wk_sb[:], wk[:, :, :])
            nc.scalar.dma_start(wq_sb[:], wq[:, :, :])
            dsl(xK, xk, 0, 0, nc.sync, kws[0])
            dsl(xK, xk, 0, 1, nc.scalar, kws[0])
            dsl(xT, xt, 0, 0, nc.sync)
            dsl(xT, xt, 0, 1, nc.scalar)
            nc.sync.dma_start(maskp[:], maskf[:, :])
            nc.sync.dma_start(bv_sb[:], bv[:, :])
            nc.scalar.dma_start(wv_sb[:], wv[:, :, :])
            dsl(xK, xk, 1, 0, nc.sync, kws[1])
            nc.sync.dma_start(bq_sb[:], bq[:, :])
            nc.sync.dma_start(bk_sb[:], bk[:, :])
            dsl(xK, xk, 1, 1, nc.scalar, kws[1])
            dsl(xK, xk, 2, 0, nc.sync, kws[2])
            dsl(xK, xk, 2, 1, nc.scalar, kws[2])
            dsl(xK, xk, 3, 0, nc.sync, kws[3])
            dsl(xK, xk, 3, 1, nc.scalar, kws[3])
            dsl(xT, xt, 1, 0, nc.sync)
            dsl(xT, xt, 1, 1, nc.scalar)
            dsl(xT, xt, 2, 0, nc.sync)
            dsl(xT, xt, 2, 1, nc.scalar)
            nc.sync.dma_start(wo_sb[:], wo[:, :, :])
            dsl(xT, xt, 3, 0, nc.sync)
            dsl(xT, xt, 3, 1, nc.scalar)

            ve4 = VE[:].rearrange("p st (h c) -> p st h c", h=HPC)

            # ---- filler units (each emits a small group of PE work) ----
            def v_unit(st):
                def emit():
                    pv = ps_w.tile([P, QW], F32, tag="w", name=f"pv{st}")
                    for dc in range(DCH):
                        nc.tensor.matmul(
                            pv[:, :256],
                            xK[:, dc, st * P : (st + 1) * P],
                            wv_sb[:, dc, :],
                            start=(dc == 0),
                            stop=(dc == DCH - 1),
                        )
                    # ve = (pv * mask) + mask*bv
                    nc.vector.scalar_tensor_tensor(
                        ve4[:, st, :, 0:DH],
                        pv[:, :256].rearrange("p (h c) -> p h c", h=HPC),
                        maskp[:, st : st + 1],
                        bvm_sb[:, st, :].rearrange("p (h c) -> p h c", h=HPC),
                        ALU.mult,
                        ALU.add,
                    )

                return emit, 2048

            def kq_unit(dst, src, w_sb, b_sb, pr, sl, w=QW, off=0):
                """Two 4-matmul halves of a K/Q projection tile."""
                qsl = slice(sl * QW + off, sl * QW + off + w)
                box = {}

                def emit_a():
                    box["pq"] = ps_w.tile([P, QW], F32, tag="w", name=f"pq{pr}_{sl}")
                    for dc in range(4):
                        nc.tensor.matmul(
                            box["pq"][:, :w],
                            w_sb[:, dc, pr * P : (pr + 1) * P],
                            src[:, dc, qsl],
                            start=(dc == 0),
                            stop=False,
                        )

                def emit_b():
                    for dc in range(4, DCH):
                        nc.tensor.matmul(
                            box["pq"][:, :w],
                            w_sb[:, dc, pr * P : (pr + 1) * P],
                            src[:, dc, qsl],
                            start=False,
                            stop=(dc == DCH - 1),
                        )
                    nc.vector.tensor_scalar_add(
                        dst[:, pr, qsl], box["pq"][:, :w], b_sb[:, pr : pr + 1]
                    )

                return (emit_a, w * 4), (emit_b, w * 4)

            ob_tiles = {}

            def out_unit(st, nt, pool=None, scalar_copy=False):
                def emit():
                    if pool is None:
                        po = ps_w.tile([P, QW], F32, tag="w", name=f"po{st}_{nt}")
                    else:
                        po = pool.tile(
                            [P, 2 * QW], F32, tag="sc", name=f"po{st}_{nt}"
                        )[:, :QW]
                    for pr in range(PAIRS):
                        nc.tensor.matmul(
                            po[:],
                            ctxT[:, pr, st * P : (st + 1) * P],
                            wo_sb[:, pr, nt * QW : (nt + 1) * QW],
                            start=(pr == 0),
                            stop=(pr == PAIRS - 1),
                        )
                    if nt == 0:
                        ob_tiles[st] = op_.tile([P, D], BF16, tag="ob", name=f"ob{st}")
                    obt = ob_tiles[st]
                    if scalar_copy:
                        nc.scalar.copy(obt[:, nt * QW : (nt + 1) * QW], po[:])
                    else:
                        nc.vector.tensor_copy(obt[:, nt * QW : (nt + 1) * QW], po[:])
                    nc.sync.dma_start(
                        out[st * P : (st + 1) * P, nt * QW : (nt + 1) * QW],
                        obt[:, nt * QW : (nt + 1) * QW],
                    )

                return emit, 1024

            # ---- attention: one software-pipelined stream, AV lagging
            # LAG chunks behind scores/exp, fillers interleaved by schedule.
            LAG = 3
            deferred = {}

            def normalize(pr, qt, hh, cp, defer=False):
                # reciprocal_approx_fast misbehaves on single-partition
                # tiles: broadcast the PSUM denominator row first, then
                # invert on the broadcast tile.
                qsl = slice(qt * QW, (qt + 1) * QW)
                den = sp.tile([1, QW], F32, tag="den", name=f"den{pr}_{qt}_{hh}")
                nc.vector.tensor_copy(den[:], cp[DH : DH + 1, :])
                denB = sp.tile([DH, QW], F32, tag="denB", name=f"denB{pr}_{qt}_{hh}")
                nc.gpsimd.partition_broadcast(denB[:], den[:])
                recB = sp.tile([DH, QW], F32, tag="recB", name=f"recB{pr}_{qt}_{hh}")
                nc.vector.reciprocal_approx_fast(recB[:], denB[:])
                if defer:
                    deferred[hh] = (cp, recB)
                else:
                    nc.vector.tensor_mul(
                        ctxT[hh * DH : (hh + 1) * DH, pr, qsl], cp[:DH, :], recB[:]
                    )

            def run_stream(schedule):
                jobs = [
                    (pr, qt, kc)
                    for pr in range(PAIRS)
                    for qt in range(NQT)
                    for kc in range(skc)
                ]
                cps_map = {}
                ets = {}
                n_gi = max(len(jobs) + LAG, max(schedule, default=0) + 1)
                for gi in range(n_gi):
                    if gi < len(jobs):
                        pr, qt, kc = jobs[gi]
                        qsl = slice(qt * QW, (qt + 1) * QW)
                        if kc == 0:
                            cps_map[(pr, qt)] = [
                                ps_ctx.tile(
                                    [P, QW], F32, tag="ctx", name=f"ctx{pr}_{qt}_{hh}"
                                )
                                for hh in range(2)
                            ]
                        sc = ps_sc.tile([P, 2 * QW], F32, tag="sc", name=f"sc{gi}")
                        for hh in range(2):
                            nc.tensor.matmul(
                                sc[:, hh * QW : (hh + 1) * QW],
                                KT[hh * DH : (hh + 1) * DH, pr, kc * P : (kc + 1) * P],
                                QT[hh * DH : (hh + 1) * DH, pr, qsl],
                                start=True,
                                stop=True,
                            )
                        et = ep.tile([P, 2 * QW], BF16, tag="et", name=f"et{gi}")
                        nc.scalar.activation(et[:], sc[:], AF.Exp)
                        ets[gi] = et
                    for f in schedule.get(gi, ()):
                        f[0]()
                    if LAG <= gi < len(jobs) + LAG:
                        pr, qt, kk = jobs[gi - LAG]
                        et = ets.pop(gi - LAG)
                        cps = cps_map[(pr, qt)]
                        for hh in range(2):
                            h = 2 * pr + hh
                            nc.tensor.matmul(
                                cps[hh][: DH + 1, :],
                                VE[:, kk, h * (DH + 1) : (h + 1) * (DH + 1)],
                                et[:, hh * QW : (hh + 1) * QW],
                                start=(kk == 0),
                                stop=(kk == skc - 1),
                            )
                            # normalize as soon as this head's accum ends
                            if kk == skc - 1:
                                normalize(pr, qt, hh, cps[hh],
                                          defer=(gi - LAG == len(jobs) - 1))
                        if kk == skc - 1:
                            del cps_map[(pr, qt)]

            # ---- emission schedule ----
            KF = lambda pr, sl: kq_unit(KT, xK, wk_sb, bk_sb, pr, sl, kws[sl])
            QF = lambda pr, qt: kq_unit(QT, xT, wq_sb, bq_sb, pr, qt)

            # prologue: K slice 0 and Q tile 0 of pair 0 only -- the first
            # V tiles go into the stream so a late wv DMA can't delay the
            # first scores.
            for f, _ in KF(0, 0):
                f()
            for f, _ in QF(0, 0):
                f()

            # bvm = maskf (x) bv built on device. Emitted AFTER the prologue
            # projections: the DVE runs in order, so putting these ~5us of
            # mask-plumbing ops (gated by the late maskp DMA) first would
            # make the K00/Q00 bias-adds -- and with them scores(0) -- wait
            # on data only V(0)/AV(0) need.
            bvB = pp.tile([P, 256], F32)
            nc.gpsimd.partition_broadcast(bvB[:], bv_sb[:])
            for st in range(skc):
                nc.vector.tensor_scalar_mul(
                    bvm_sb[:, st, :], bvB[:], maskp[:, st : st + 1]
                )
            # mask columns of V_ext (disjoint from the V column writes)
            nc.vector.tensor_copy(
                ve4[:, :, :, DH : DH + 1],
                maskp[:, :, None, None].to_broadcast([P, skc, HPC, 1]),
            )

            schedule = {}

            def put(gi, *units):
                for u in units:
                    schedule.setdefault(gi, []).append(u)
                    gi += 1

            # tile 0: remaining K slices at 4s-2 (needed by scores at 4s),
            # V tiles greedily at <=1 unit/gi alongside them, <=2 otherwise,
            # all before their AV consumes them (gi j+LAG-1); Q(0,1) last.
            for s2 in range(1, nsl):
                put(4 * s2 - 2, *KF(0, s2))
            cur = 0
            for j in range(skc):
                while len(schedule.get(cur, [])) >= (
                    1 if any(u[1] > 2048 for u in schedule.get(cur, [])) else 2
                ):
                    cur += 1
                assert cur <= j + LAG - 1, (j, cur)
                put(cur, v_unit(j))
            put(skc - 2, *QF(0, 1))
            # tiles 1-3: Q for upcoming tiles, K for pair 1
            put(skc + 8, *QF(0, 2))
            put(2 * skc + 3, *KF(1, 0))
            put(2 * skc + 7, *KF(1, 1))
            put(2 * skc + 11, *QF(0, 3))
            put(3 * skc + 4, *KF(1, 2))
            put(3 * skc + 8, *KF(1, 3))
            put(3 * skc + 12, *QF(1, 0))
            # tiles 4-5: Q for pair 1's later tiles
            put(4 * skc + 4, *QF(1, 1))
            put(4 * skc + 9, *QF(1, 2))
            put(6 * skc + 11, *QF(1, 3))
            # out projections as (1,qt) tiles complete (bases sit a few gi
            # after the tile's normalize chain so the po matmuls never wait
            # on the DVE queue)
            for qt, base in ((0, 5 * skc + 7), (1, 6 * skc + 7), (2, 7 * skc + 5)):
                for i in range(4):
                    st = 4 * qt + i
                    schedule.setdefault(base + 2 * i, []).append(out_unit(st, 0))
                    schedule.setdefault(base + 8 + 2 * i, []).append(out_unit(st, 1))

            run_stream(schedule)

            # epilogue: last q-tile's output projection; ctx normalize runs
            # per 128-column slice just ahead of each out projection, and po
            # accumulators rotate through the idle sc ring.
            j = 0
            for st in range(12, 16):
                lo = (st - 12) * P
                for hh in range(2):
                    cp, recB = deferred[hh]
                    nc.vector.tensor_mul(
                        ctxT[hh * DH : (hh + 1) * DH, 1, st * P : (st + 1) * P],
                        cp[:DH, lo : lo + P],
                        recB[:, lo : lo + P],
                    )
                for nt in range(2):
                    out_unit(
                        st, nt, pool=None if j % 3 == 2 else ps_sc,
                        scalar_copy=True,
                    )[0]()
                    j += 1

    nc.finalize()
    return nc


def _pack_keys(xb, maskf_b):
    """Pack kept keys of one batch; returns (xk rows [nk, D], nk)."""
    kept = np.flatnonzero(maskf_b > 0.5)
    return xb[kept], len(kept)


def shard_inputs(x, Wq, bq, Wk, bk, Wv, bv, Wo, bo, mask):
    """Full inputs -> (skc, list of 8 per-core input maps)."""
    maskf = (~np.asarray(mask)).astype(np.float32)  # 1.0 = keep
    x = np.asarray(x, dtype=np.float32)
    Wq, Wk, Wv, Wo = (np.asarray(w, dtype=np.float32) for w in (Wq, Wk, Wv, Wo))
    bq, bk, bv = (np.asarray(b, dtype=np.float32) for b in (bq, bk, bv))

    packed = [_pack_keys(x[b], maskf[b]) for b in range(2)]
    skc = max(13, max(math.ceil(nk / P) for _, nk in packed))
    skc = min(skc, S // P)
    SKW = skc * P

    def dev3(w):  # [1024, 256] -> [128, 8, 256] bf16
        return np.ascontiguousarray(
            w.reshape(DCH, P, 256).transpose(1, 0, 2).astype(BF16NP)
        )

    per_batch = []
    for b in range(2):
        xk_rows, nk = packed[b]
        if nk > SKW:  # mask denser than expected: fall back to unpacked
            xk_rows, nk = x[b], S
        xk_full = np.zeros((SKW, D), np.float32)
        xk_full[:nk] = xk_rows
        xkd = np.ascontiguousarray(
            xk_full.T.reshape(DCH, P, SKW).transpose(1, 0, 2).astype(BF16NP)
        )
        xtd = np.ascontiguousarray(
            x[b].T.reshape(DCH, P, S).transpose(1, 0, 2).astype(BF16NP)
        )
        mp = (np.arange(SKW).reshape(skc, P).T < nk).astype(np.float32)
        per_batch.append((xtd, xkd, np.ascontiguousarray(mp)))

    ins = []
    for c in range(N_CORES):
        b, g = divmod(c, 4)
        cs = slice(g * 256, (g + 1) * 256)
        xtd, xkd, mp = per_batch[b]
        wo_d = np.ascontiguousarray(
            Wo[cs, :].reshape(PAIRS, P, D).transpose(1, 0, 2).astype(BF16NP)
        )
        ins.append(
            {
                "xt": xtd,
                "xk": xkd,
                "wq": dev3(Wq[:, cs]),
                "wk": dev3(Wk[:, cs] * SCALE),
                "wv": dev3(Wv[:, cs]),
                "wo": wo_d,
                "bq": np.ascontiguousarray(bq[cs].reshape(PAIRS, P).T),
                "bk": np.ascontiguousarray(bk[cs].reshape(PAIRS, P).T * SCALE),
                "bv": np.ascontiguousarray(bv[None, cs]),
                "maskf": mp,
            }
        )
    return skc, ins


def gather_outputs(results, bo):
    """8 per-core partial outputs (bf16) -> full (2, S, D) fp32 output."""
    outs = []
    for b in range(2):
        acc = results[4 * b]["out"].astype(np.float32)
        for g in range(1, 4):
            acc += results[4 * b + g]["out"].astype(np.float32)
        outs.append(acc + np.asarray(bo, dtype=np.float32))
    return np.stack(outs, axis=0)


_NC_CACHE = {}


def _get_nc(skc):
    if skc not in _NC_CACHE:
        _NC_CACHE[skc] = build(skc)
    return _NC_CACHE[skc]


def run_sharded(inputs, trace=False, tmpdir=None):
    """Shard, run on cores 0-7, gather. Returns (output, BassKernelResults)."""
    skc, ins = shard_inputs(**inputs)
    nc = _get_nc(skc)
    res = run_bass_kernel_spmd(
        nc, ins, core_ids=list(range(N_CORES)), trace=trace, tmpdir=tmpdir
    )
    full = gather_outputs(res.results, inputs["bo"])
    return full, res


def kernel(**inputs) -> np.ndarray:
    full, _ = run_sharded(inputs, trace=False)
    return full
